# revision 12
# baseline (speedup 1.0000x reference)
"""BitLinear (activation int8-quant + ternary weight) Trainium2 kernel.

Strategy (8 NeuronCores, token-parallel, fp8 DoubleRow GEMM):
  - x [2,8192,2048] -> flat [16384, 2048]; core c gets a contiguous slice of
    2048 tokens (natural [token, feature] layout).
  - Weight quantization is a pure function of the static weight, so it is done
    host-side (BitNet ships ternary weights): w_q = ternary(W) with the same
    f32 compares as the reference, shipped pre-transposed as fp8e4 planes
    (16*w_q, w_q) interleaved per k-tile -> 8.4MB/core instead of 16.8MB f32,
    and zero device-side W math.
  - Activation quant on device, exact int8: DVE abs-max per token row ->
    127/s scale; ACT magic-round (+1.5*2^23) -> x_q bf16 ints in [-127,127].
  - x_q is split exactly as x_q = 16*hi + lo with hi,lo in [-8,8]: both are
    exact in fp8e4 (e4m3).  hi = rne(x_q/16) via the bf16 magic constant 192
    (ulp=1 in [128,256)), lo = x_q - 16*hi.  Paired with weight planes
    (16*w, w) on the doubled contraction axis of a DoubleRow fp8 matmul:
      psum += hi.T @ (16 w) + lo.T @ w  ==  x_q.T @ w   exactly (f32 psum,
    |acc| <= 127*2048 < 2^24).  DoubleRow runs fp8 pairs at 2x bf16 rate.
  - Postprocess: DVE relu(gf*acc) (gf = w_scale/s per token), ACT Square,
    DMA out f32.
"""

import sys

if "/opt/trn_rl_repo" not in sys.path:
    sys.path.insert(0, "/opt/trn_rl_repo")

import numpy as np

N_CORES = 8
P = 128
TOK_TOTAL = 16384
TOK = TOK_TOTAL // N_CORES  # 2048 tokens per core
D_IN = 2048
D_OUT = 2048
NK = D_IN // P  # 16 contraction tiles
NM = TOK // P  # 16 token blocks per core
NCHUNK = 512  # psum bank free dim (f32)
NN = D_OUT // NCHUNK  # 4
# float32 round-to-nearest-even integer trick: adding 1.5*2^23 puts any
# value in [-2^22, 2^22] into [2^23, 2^24) where the f32 ulp is exactly 1,
# so the add rounds RNE to an integer; subtracting recovers round(x).
MAGIC = 12582912.0  # 1.5 * 2**23
# bf16 variant for values in [-64, 64]: 1.5*2^7 puts them in [128, 256)
# where the bf16 ulp is 1.
BMAGIC = 192.0

_tile_patched = False


def _patch_tile_drain():
    """walrus in this container rejects >2 sem waits on the TileContext exit
    Drain ("Too many sync wait commands").  Split the excess waits onto
    explicit SP wait_ge instructions (same semantics: all waits complete
    before the semaphore free + final barrier)."""
    global _tile_patched
    if _tile_patched:
        return
    import concourse.tile as tile
    from bass_rust import ScopedClock

    def patched(self, tick_clock, wait_clock):
        nc_ = self.nc
        drain_inst = nc_.sync.drain()
        wait_clock.add_sem_waits(
            drain_inst.ins, ScopedClock({None: tick_clock.global_clock})
        )
        waits = list(drain_inst.ins.sync_info.on_wait or [])
        if len(waits) > 1:
            drain_inst.ins.sync_info.on_wait = waits[:1]
            name_to_sem = {}
            for key, h in self.sems.allocated().items():
                name_to_sem[getattr(h, "name", str(key))] = h
            for w in waits[1:]:
                nc_.sync.wait_ge(name_to_sem[w.ant_name], w.wait_value)
        nc_.all_engine_barrier()
        popped = nc_._tile_sem_poison_stack.pop()
        assert popped is self._sem_poison
        nc_.clear_and_free_semaphores(list(self.sems.allocated().values()))
        nc_.all_engine_barrier()

    tile.TileContext._drain_and_barrier = patched
    _tile_patched = True


def _split_excess_waits(nc, max_waits: int = 1):
    """walrus's setupSyncWait caps the number of semaphore waits a single
    instruction can carry.  Tile's scheduler freely attaches more.  Move the
    excess onto wait-only EventSemaphore carrier instructions inserted just
    before the over-subscribed instruction on the same engine (program order
    on one engine => identical semantics)."""
    from concourse import mybir

    n_split = 0
    for fn in nc.m.functions:
        for bb in fn.blocks:
            insts = bb.instructions
            i = 0
            while i < len(insts):
                inst = insts[i]
                si = getattr(inst, "sync_info", None)
                waits = list(si.on_wait) if (si is not None and si.on_wait) else []
                # The ucode DMA-transpose path does not reliably honor
                # instruction-level sem waits -> move ALL of its waits onto
                # engine-level carriers so the sequencer blocks before
                # pushing the transpose.  Same for matmul: walrus splits the
                # self-loading InstMatmult into LDWEIGHTS+MATMUL with the
                # waits on the MATMUL, so the LDWEIGHTS (which reads the
                # stationary operand) can execute before the wait is honored
                # - the carrier blocks the PE sequencer before both.
                limit = (
                    0
                    if type(inst).__name__ in ("InstDmaTransposeAnt", "InstMatmult")
                    else max_waits
                )
                if len(waits) <= limit:
                    i += 1
                    continue
                keep = waits[-limit:] if limit else []
                extras = waits[: len(waits) - limit]
                pos = i
                for j in range(0, len(extras), max_waits):
                    ev = mybir.InstEventSemaphore(
                        name=f"wsplit_{inst.name}_{j}_{n_split}",
                        engine=inst.engine,
                        ins=[],
                        outs=[],
                        sync_info=mybir.SyncInfo(
                            on_wait=extras[j : j + max_waits], on_update=[]
                        ),
                    )
                    try:
                        nc.register_instruction(ev, overwrite=True)
                    except Exception:
                        pass
                    insts.insert(pos, ev)
                    pos += 1
                inst.sync_info.on_wait = keep
                n_split += 1
                i = pos + 1
    return n_split


def build_program(w_scale: float):
    """Build the per-core Bass program (same program runs SPMD on all 8
    cores; per-core data arrives via the input map)."""
    import concourse.bass as bass
    import concourse.tile as tile
    from concourse import mybir

    f32 = mybir.dt.float32
    bf16 = mybir.dt.bfloat16
    f8 = mybir.dt.float8e4
    AF = mybir.ActivationFunctionType
    ALU = mybir.AluOpType
    AX = mybir.AxisListType
    PM = mybir.MatmulPerfMode

    _patch_tile_drain()

    ws_f32 = float(np.float32(w_scale))

    nc = bass.Bass("TRN2", target_bir_lowering=False, debug=False)
    xs = nc.dram_tensor("xs", [TOK, D_IN], f32, kind="ExternalInput").ap()
    # host-ternarized fp8 weight planes: row k*128+p, cols [j*2048+o] with
    # plane j=0 holding 16*w_q^T and j=1 holding w_q^T
    wq = nc.dram_tensor("wq8", [NK * P, 2 * D_OUT], f8, kind="ExternalInput").ap()
    ident = nc.dram_tensor("ident", [P, P], bf16, kind="ExternalInput").ap()
    y = nc.dram_tensor("y", [TOK, D_OUT], f32, kind="ExternalOutput").ap()
    import os as _os

    _dup = _os.environ.get("KDUP") == "1"
    if _dup:
        y2 = nc.dram_tensor("y2", [TOK, D_OUT], f32, kind="ExternalOutput").ap()

    with tile.TileContext(nc) as tc:
        with (
            tc.tile_pool(name="wq", bufs=1) as wq_pool,
            tc.tile_pool(name="xin", bufs=(2 if _dup else 3)) as x_pool,
            tc.tile_pool(name="xq", bufs=(3 if _dup else 5)) as xq_pool,
            tc.tile_pool(name="hib", bufs=2) as hi_pool,
            tc.tile_pool(name="q8", bufs=(16 if _dup else 4)) as q8_pool,
            tc.tile_pool(name="scal", bufs=18) as s_pool,
            tc.tile_pool(name="psum", bufs=4, space="PSUM") as psum_pool,
            tc.tile_pool(name="pstr", bufs=2, space="PSUM") as tr_pool,
            tc.tile_pool(name="outa", bufs=(1 if _dup else 2)) as a_pool,
            tc.tile_pool(name="outb", bufs=(1 if _dup else 2)) as b_pool,
            tc.tile_pool(name="consts", bufs=1) as c_pool,
        ):
            # persistent fp8 weight planes [p, k, j, o]
            wq8 = wq_pool.tile([P, NK, 2, D_OUT], f8)
            cmagic = c_pool.tile([P, 1], f32)
            nc.vector.memset(cmagic[:], MAGIC)
            idt = c_pool.tile([P, P], bf16)
            nc.sync.dma_start(idt[:], ident[:, :])

            gfs = {}
            xqs = {}
            q8s = {}

            def emit_w(k):
                nc.sync.dma_start(wq8[:, k, :, :], wq[k * P : (k + 1) * P, :])

            def emit_x(m):
                xf = x_pool.tile([P, D_IN], f32, tag="xf", name=f"xf_{m}")
                nc.sync.dma_start(xf[:], xs[m * P : (m + 1) * P, :])
                s0 = s_pool.tile([P, 1], f32, tag="s0", name=f"s0_{m}")
                nc.vector.tensor_reduce(
                    s0[:], xf[:], AX.X, ALU.max, apply_absolute_value=True
                )
                s1 = s_pool.tile([P, 1], f32, tag="s1", name=f"s1_{m}")
                nc.vector.tensor_scalar(s1[:], s0[:], 1e-5, None, ALU.max)
                rf = s_pool.tile([P, 1], f32, tag="rf", name=f"rf_{m}")
                nc.vector.reciprocal(rf[:], s1[:])
                qf = s_pool.tile([P, 1], f32, tag="qf", name=f"qf_{m}")
                nc.vector.tensor_scalar(qf[:], rf[:], 127.0, None, ALU.mult)
                gf = s_pool.tile([P, 1], f32, tag="gf", name=f"gf_{m}")
                nc.vector.tensor_scalar(gf[:], rf[:], ws_f32, None, ALU.mult)
                gfs[m] = gf
                # x_q = round(x * 127/s): magic add on ACT (in place over xf),
                # magic subtract + bf16 cast on DVE
                nc.scalar.activation(
                    xf[:], xf[:], AF.Identity, bias=cmagic[:, 0:1], scale=qf[:, 0:1]
                )
                xq = xq_pool.tile([P, D_IN], bf16, tag="xq", name=f"xq_{m}")
                nc.vector.tensor_scalar(xq[:], xf[:], MAGIC, None, ALU.subtract)
                xqs[m] = xq

            def transform(m):
                # PE transpose (identity matmul) into PSUM: avoids the ucode
                # DMA-transpose path whose completion signaling races with
                # DVE consumers.  trp[p_i, k, t] = xq[t, 128k+p]
                xq = xqs[m]
                trp = tr_pool.tile([P, NK, P], bf16, tag="tr", name=f"tr_{m}")
                for k in range(NK):
                    nc.tensor.transpose(
                        trp[:, k, :], xq[:, k * P : (k + 1) * P], idt[:]
                    )
                # exact split x_q = 16*hi + lo, planes -> fp8
                # t1 = rne(x_q/16) + 192 via bf16 magic (ulp 1 in [128,256))
                t1 = hi_pool.tile([P, D_IN], bf16, tag="t1", name=f"t1_{m}")
                nc.vector.tensor_scalar(
                    t1[:], trp[:, :, :], 1.0 / 16.0, BMAGIC, ALU.mult, ALU.add
                )
                q8 = q8_pool.tile([P, NK, 2, P], f8, tag="q8", name=f"q8_{m}")
                t1_v = t1[:].rearrange("p (k t) -> p k t", k=NK)
                # hi plane (j=0): t1 - 192
                nc.vector.tensor_scalar(
                    q8[:, :, 0, :], t1_v, BMAGIC, None, ALU.subtract
                )
                # hi16 = (t1 - 192) * 16 (exact in bf16, |.| <= 128)
                hi16 = hi_pool.tile([P, D_IN], bf16, tag="hi16", name=f"hi16_{m}")
                nc.vector.tensor_scalar(
                    hi16[:], t1[:], BMAGIC, 16.0, ALU.subtract, ALU.mult
                )
                # lo plane (j=1): x_q - 16*hi
                hi16_v = hi16[:].rearrange("p (k t) -> p k t", k=NK)
                nc.vector.tensor_tensor(
                    q8[:, :, 1, :], trp[:, :, :], hi16_v, ALU.subtract
                )
                q8s[m] = q8

            # Emission order = Tile priority.  x0/x1 chains first (earliest
            # PE ramp work), then the W DMA burst (pure DMA now - cheap)
            # interleaved with the remaining x blocks.
            emit_x(0)
            emit_x(1)
            transform(0)
            for k in range(NK):
                emit_w(k)
                if 2 + k < NM:
                    emit_x(2 + k)
            transform(1)

            # ---- Phase 2: fp8 DoubleRow gemm + postprocess per token block,
            # with the PE transpose of block m+2 pipelined behind block m's
            # matmuls.
            for m in range(NM):
                q8 = q8s[m]
                gf = gfs[m]
                psums = []
                for n in range(NN):
                    ps = psum_pool.tile([P, NCHUNK], f32, tag="ps", name=f"ps_{m}_{n}")
                    psums.append(ps)
                for k in range(NK):
                    for n in range(NN):
                        nc.tensor.matmul(
                            psums[n][:],
                            q8[:, k, :, :],
                            wq8[:, k, :, n * NCHUNK : (n + 1) * NCHUNK],
                            start=(k == 0),
                            stop=(k == NK - 1),
                            perf_mode=PM.DoubleRow,
                        )

                # out = (gf * relu(acc))^2 : relu+scale on DVE, square on ACT
                A = a_pool.tile([P, D_OUT], f32, tag="A", name=f"A_{m}")
                for n in range(NN):
                    nc.vector.tensor_scalar(
                        A[:, n * NCHUNK : (n + 1) * NCHUNK],
                        psums[n][:],
                        gf[:, 0:1],
                        0.0,
                        ALU.mult,
                        ALU.max,
                    )
                B = b_pool.tile([P, D_OUT], f32, tag="B", name=f"B_{m}")
                for n in range(NN):
                    src = A[:, n * NCHUNK : (n + 1) * NCHUNK]
                    dst = B[:, n * NCHUNK : (n + 1) * NCHUNK]
                    nc.scalar.activation(dst, src, AF.Square)
                nc.sync.dma_start(y[m * P : (m + 1) * P, :], B[:])
                if m + 2 < NM:
                    transform(m + 2)

            if _dup:
                # diagnostic second GEMM pass over the same q8 tiles, emitted
                # after everything else (inputs long settled)
                for m in range(NM):
                    q8 = q8s[m]
                    gf = gfs[m]
                    psums2 = []
                    for n in range(NN):
                        ps = psum_pool.tile(
                            [P, NCHUNK], f32, tag="ps", name=f"ps2_{m}_{n}"
                        )
                        psums2.append(ps)
                    for k in range(NK):
                        for n in range(NN):
                            nc.tensor.matmul(
                                psums2[n][:],
                                q8[:, k, :, :],
                                wq8[:, k, :, n * NCHUNK : (n + 1) * NCHUNK],
                                start=(k == 0),
                                stop=(k == NK - 1),
                                perf_mode=PM.DoubleRow,
                            )
                    A2 = a_pool.tile([P, D_OUT], f32, tag="A", name=f"A2_{m}")
                    for n in range(NN):
                        nc.vector.tensor_scalar(
                            A2[:, n * NCHUNK : (n + 1) * NCHUNK],
                            psums2[n][:],
                            gf[:, 0:1],
                            0.0,
                            ALU.mult,
                            ALU.max,
                        )
                    B2 = b_pool.tile([P, D_OUT], f32, tag="B", name=f"B2_{m}")
                    for n in range(NN):
                        nc.scalar.activation(
                            B2[:, n * NCHUNK : (n + 1) * NCHUNK],
                            A2[:, n * NCHUNK : (n + 1) * NCHUNK],
                            AF.Square,
                        )
                    nc.sync.dma_start(y2[m * P : (m + 1) * P, :], B2[:])

    _split_excess_waits(nc)
    return nc


def _w_scale_like_reference(weight: np.ndarray) -> float:
    """mean(|W|) computed with jax on CPU so it is bit-identical to the
    reference's jnp.mean(jnp.abs(weight))."""
    try:
        import jax
        import jax.numpy as jnp

        cpu = jax.devices("cpu")[0]
        with jax.default_device(cpu):
            return float(jnp.mean(jnp.abs(jnp.asarray(weight, dtype=jnp.float32))))
    except Exception:
        return float(np.float32(np.abs(weight).astype(np.float64).mean()))


def _ternarize_host(weight: np.ndarray, w_scale: float):
    """Ternary weight planes as fp8e4, matching the reference's f32
    compares: w_q = 1[w > 0.5*ws] - 1[w < -0.5*ws]."""
    import ml_dtypes

    f8 = ml_dtypes.float8_e4m3
    w = weight.astype(np.float32, copy=False)
    thr = np.float32(0.5) * np.float32(w_scale)
    wq = (w > thr).astype(np.float32) - (w < -thr).astype(np.float32)
    wqT = np.ascontiguousarray(wq.T)  # [in, out]
    wk = wqT.reshape(NK, P, D_OUT)
    arr = np.empty((NK, P, 2, D_OUT), dtype=f8)
    arr[:, :, 0, :] = (np.float32(16.0) * wk).astype(f8)
    arr[:, :, 1, :] = wk.astype(f8)
    return np.ascontiguousarray(arr.reshape(NK * P, 2 * D_OUT))


def make_in_maps(x: np.ndarray, weight: np.ndarray, w_scale: float | None = None):
    import ml_dtypes

    if w_scale is None:
        w_scale = _w_scale_like_reference(weight)
    x_flat = np.ascontiguousarray(
        x.reshape(TOK_TOTAL, D_IN).astype(np.float32, copy=False)
    )
    wq8 = _ternarize_host(weight, w_scale)
    ident = np.eye(P, dtype=ml_dtypes.bfloat16)
    return [
        {"xs": x_flat[c * TOK : (c + 1) * TOK, :], "wq8": wq8, "ident": ident}
        for c in range(N_CORES)
    ]


def run_on_hw(x: np.ndarray, weight: np.ndarray, trace: bool = False):
    """Compile + execute on the 8 NeuronCores.  Returns (y_full, results)."""
    from concourse.bass_utils import run_bass_kernel_spmd

    if trace:
        _install_ntff_hook()
    w_scale = _w_scale_like_reference(weight)
    nc = build_program(w_scale)
    in_maps = make_in_maps(x, weight, w_scale)
    res = run_bass_kernel_spmd(nc, in_maps, list(range(N_CORES)), trace=trace)
    y_full = np.concatenate(
        [np.asarray(res.results[c]["y"]) for c in range(N_CORES)], axis=0
    ).reshape(x.shape[0], x.shape[1], D_OUT)
    return y_full.astype(np.float32, copy=False), res


def _install_ntff_hook():
    """The agent image's antenv package lacks axon_hooks, so NTFF profiling
    silently degrades.  Recreate the hook module (ctypes into
    libaxon_pjrt.so) so run_bass_kernel_spmd(trace=True) works."""
    import types, ctypes, contextlib, os

    if "antenv.axon_hooks" in sys.modules:
        return
    so_path = "/opt/axon/libaxon_pjrt.so"
    if not os.path.exists(so_path):
        return
    lib = ctypes.CDLL(so_path)
    if not hasattr(lib, "axon_start_nrt_profile"):
        return
    lib.axon_start_nrt_profile.argtypes = [
        ctypes.POINTER(ctypes.c_int64),
        ctypes.c_size_t,
    ]
    lib.axon_start_nrt_profile.restype = ctypes.c_int64
    lib.axon_stop_nrt_profile.argtypes = [ctypes.c_char_p]
    lib.axon_stop_nrt_profile.restype = ctypes.c_int64

    @contextlib.contextmanager
    def _hook(output_dir, device_ids):
        import jax

        jax.devices()
        if device_ids:
            ids = (ctypes.c_int64 * len(device_ids))(*device_ids)
            rc = lib.axon_start_nrt_profile(ids, len(device_ids))
        else:
            rc = lib.axon_start_nrt_profile(None, 0)
        if rc != 0:
            raise RuntimeError(f"axon_start_nrt_profile rc={rc}")
        try:
            yield
        finally:
            n = lib.axon_stop_nrt_profile(str(output_dir).encode())
            print(f"profile: {n} file(s) written to {output_dir}", file=sys.stderr)

    mod = types.ModuleType("antenv.axon_hooks")
    mod.get_axon_ntff_profile_hook = lambda: _hook
    mod.set_axon_ntff_profile_hook = lambda h: None
    sys.modules["antenv.axon_hooks"] = mod

    # upload_artifacts needs a coo bucket this container doesn't have;
    # degrade to a no-op so trace processing can proceed locally.
    import concourse.bass_utils as bu

    _orig_upload = bu.upload_artifacts

    def _safe_upload(tmpdir):
        try:
            return _orig_upload(tmpdir)
        except Exception as e:
            print(f"upload_artifacts skipped: {e}", file=sys.stderr)
            return tmpdir

    bu.upload_artifacts = _safe_upload


def kernel(x: np.ndarray, weight: np.ndarray) -> np.ndarray:
    y, _ = run_on_hw(x, weight, trace=False)
    return y


# revision 13
# speedup vs baseline: 1.2047x; 1.2047x over previous
"""BitLinear (activation int8-quant + ternary weight) Trainium2 kernel.

Strategy (8 NeuronCores, token-parallel, bf16 GEMM):
  - x [2,8192,2048] -> flat [16384, 2048]; core c gets a contiguous slice of
    2048 tokens (natural [token, feature] layout).
  - Weight quantization is a pure function of the static weight, so it is done
    host-side (BitNet ships ternary weights): w_q = ternary(W) with the same
    f32 compares as the reference, shipped pre-transposed as bf16
    (8.4MB/core instead of 16.8MB f32) with zero device-side W math.  This
    removes the baseline's 51us DVE ternarize chain that starved the PE and
    caused HAM re-throttling.
  - Activation quant on device, exact int8: DVE abs-max per token row ->
    127/s scale; ACT magic-round (+1.5*2^23) -> x_q bf16 ints in [-127,127];
    DMA-xbar transposes 128x128 bf16 tiles -> x_q^T (consumed ONLY by the
    PE: the ucode transpose's completion signaling races with DVE
    consumers, but the PE path is proven reliable).
  - PE bf16 matmuls (K=128, N=512) accumulate exactly in fp32 PSUM
    (|acc| <= 127*2048 < 2^24).  fp8 DoubleRow was measured at 259ns per
    K=256xN=512 (1.64x bf16) - a hi/lo int8 split doubles K, making it a
    net 1.22x LOSS, so bf16 is optimal here.
  - Postprocess per 512-chunk, interleaved so PSUM banks free early:
    ACT Relu(acc * ws/s), ACT Square, chunked DMA out f32.
"""

import sys

if "/opt/trn_rl_repo" not in sys.path:
    sys.path.insert(0, "/opt/trn_rl_repo")

import numpy as np

N_CORES = 8
P = 128
TOK_TOTAL = 16384
TOK = TOK_TOTAL // N_CORES  # 2048 tokens per core
D_IN = 2048
D_OUT = 2048
NK = D_IN // P  # 16 contraction tiles
NM = TOK // P  # 16 token blocks per core
NCHUNK = 512  # psum bank free dim (f32)
NN = D_OUT // NCHUNK  # 4
# float32 round-to-nearest-even integer trick: adding 1.5*2^23 puts any
# value in [-2^22, 2^22] into [2^23, 2^24) where the f32 ulp is exactly 1,
# so the add rounds RNE to an integer; subtracting recovers round(x).
MAGIC = 12582912.0  # 1.5 * 2**23

_tile_patched = False


def _patch_tile_drain():
    """walrus in this container rejects >2 sem waits on the TileContext exit
    Drain ("Too many sync wait commands").  Split the excess waits onto
    explicit SP wait_ge instructions (same semantics: all waits complete
    before the semaphore free + final barrier)."""
    global _tile_patched
    if _tile_patched:
        return
    import concourse.tile as tile
    from bass_rust import ScopedClock

    def patched(self, tick_clock, wait_clock):
        nc_ = self.nc
        drain_inst = nc_.sync.drain()
        wait_clock.add_sem_waits(
            drain_inst.ins, ScopedClock({None: tick_clock.global_clock})
        )
        waits = list(drain_inst.ins.sync_info.on_wait or [])
        if len(waits) > 1:
            drain_inst.ins.sync_info.on_wait = waits[:1]
            name_to_sem = {}
            for key, h in self.sems.allocated().items():
                name_to_sem[getattr(h, "name", str(key))] = h
            for w in waits[1:]:
                nc_.sync.wait_ge(name_to_sem[w.ant_name], w.wait_value)
        nc_.all_engine_barrier()
        popped = nc_._tile_sem_poison_stack.pop()
        assert popped is self._sem_poison
        nc_.clear_and_free_semaphores(list(self.sems.allocated().values()))
        nc_.all_engine_barrier()

    tile.TileContext._drain_and_barrier = patched
    _tile_patched = True


def _split_excess_waits(nc, max_waits: int = 1):
    """walrus's setupSyncWait caps the number of semaphore waits a single
    instruction can carry.  Tile's scheduler freely attaches more.  Move the
    excess onto wait-only EventSemaphore carrier instructions inserted just
    before the over-subscribed instruction on the same engine (program order
    on one engine => identical semantics)."""
    from concourse import mybir

    n_split = 0
    for fn in nc.m.functions:
        for bb in fn.blocks:
            insts = bb.instructions
            i = 0
            while i < len(insts):
                inst = insts[i]
                si = getattr(inst, "sync_info", None)
                waits = list(si.on_wait) if (si is not None and si.on_wait) else []
                # The ucode DMA-transpose path does not reliably honor
                # instruction-level sem waits -> move ALL of its waits onto
                # engine-level carriers so the sequencer blocks before
                # pushing the transpose.  Same for matmul: walrus splits the
                # self-loading InstMatmult into LDWEIGHTS+MATMUL with the
                # waits on the MATMUL, so the LDWEIGHTS (which reads the
                # stationary operand) can execute before the wait is honored
                # - the carrier blocks the PE sequencer before both.
                limit = (
                    0
                    if type(inst).__name__ in ("InstDmaTransposeAnt", "InstMatmult")
                    else max_waits
                )
                if len(waits) <= limit:
                    i += 1
                    continue
                keep = waits[-limit:] if limit else []
                extras = waits[: len(waits) - limit]
                pos = i
                for j in range(0, len(extras), max_waits):
                    ev = mybir.InstEventSemaphore(
                        name=f"wsplit_{inst.name}_{j}_{n_split}",
                        engine=inst.engine,
                        ins=[],
                        outs=[],
                        sync_info=mybir.SyncInfo(
                            on_wait=extras[j : j + max_waits], on_update=[]
                        ),
                    )
                    try:
                        nc.register_instruction(ev, overwrite=True)
                    except Exception:
                        pass
                    insts.insert(pos, ev)
                    pos += 1
                inst.sync_info.on_wait = keep
                n_split += 1
                i = pos + 1
    return n_split


def build_program(w_scale: float):
    """Build the per-core Bass program (same program runs SPMD on all 8
    cores; per-core data arrives via the input map)."""
    import concourse.bass as bass
    import concourse.tile as tile
    from concourse import mybir

    f32 = mybir.dt.float32
    bf16 = mybir.dt.bfloat16
    AF = mybir.ActivationFunctionType
    ALU = mybir.AluOpType
    AX = mybir.AxisListType

    _patch_tile_drain()

    ws_f32 = float(np.float32(w_scale))

    nc = bass.Bass("TRN2", target_bir_lowering=False, debug=False)
    xs = nc.dram_tensor("xs", [TOK, D_IN], f32, kind="ExternalInput").ap()
    # host-ternarized bf16 W^T: row i (input feature), col o
    wq = nc.dram_tensor("wqt", [D_IN, D_OUT], bf16, kind="ExternalInput").ap()
    y = nc.dram_tensor("y", [TOK, D_OUT], f32, kind="ExternalOutput").ap()

    with tile.TileContext(nc) as tc:
        with (
            tc.tile_pool(name="wq", bufs=1) as wq_pool,
            tc.tile_pool(name="xin", bufs=3) as x_pool,
            tc.tile_pool(name="xq", bufs=2) as xq_pool,
            tc.tile_pool(name="xqt", bufs=4) as xqt_pool,
            tc.tile_pool(name="scal", bufs=18) as s_pool,
            tc.tile_pool(name="psum", bufs=8, space="PSUM") as psum_pool,
            tc.tile_pool(name="outa", bufs=2) as a_pool,
            tc.tile_pool(name="outb", bufs=2) as b_pool,
            tc.tile_pool(name="consts", bufs=1) as c_pool,
        ):
            # persistent bf16 ternary W^T [p, k, o]
            wq8 = wq_pool.tile([P, NK, D_OUT], bf16)
            cmagic = c_pool.tile([P, 1], f32)
            nc.vector.memset(cmagic[:], MAGIC)

            gfs = {}
            xqts = {}

            def emit_w(k):
                nc.sync.dma_start(wq8[:, k, :], wq[k * P : (k + 1) * P, :])

            def emit_x(m):
                xf = x_pool.tile([P, D_IN], f32, tag="xf", name=f"xf_{m}")
                nc.sync.dma_start(xf[:], xs[m * P : (m + 1) * P, :])
                s0 = s_pool.tile([P, 1], f32, tag="s0", name=f"s0_{m}")
                nc.vector.tensor_reduce(
                    s0[:], xf[:], AX.X, ALU.max, apply_absolute_value=True
                )
                s1 = s_pool.tile([P, 1], f32, tag="s1", name=f"s1_{m}")
                nc.vector.tensor_scalar(s1[:], s0[:], 1e-5, None, ALU.max)
                rf = s_pool.tile([P, 1], f32, tag="rf", name=f"rf_{m}")
                nc.vector.reciprocal(rf[:], s1[:])
                qf = s_pool.tile([P, 1], f32, tag="qf", name=f"qf_{m}")
                nc.vector.tensor_scalar(qf[:], rf[:], 127.0, None, ALU.mult)
                gf = s_pool.tile([P, 1], f32, tag="gf", name=f"gf_{m}")
                nc.vector.tensor_scalar(gf[:], rf[:], ws_f32, None, ALU.mult)
                gfs[m] = gf
                # x_q = round(x * 127/s): magic add on ACT (in place over xf),
                # magic subtract + bf16 cast on DVE
                nc.scalar.activation(
                    xf[:], xf[:], AF.Identity, bias=cmagic[:, 0:1], scale=qf[:, 0:1]
                )
                xq = xq_pool.tile([P, D_IN], bf16, tag="xq", name=f"xq_{m}")
                nc.vector.tensor_scalar(xq[:], xf[:], MAGIC, None, ALU.subtract)
                # one 3D xbar transpose writes all 16 k-tiles:
                # xqt[p, k, t] = xq[t, 128k+p].  Consumed ONLY by PE.
                xqt = xqt_pool.tile([P, D_IN], bf16, tag="xqt", name=f"xqt_{m}")
                eng = nc.sync if (m % 2 == 0) else nc.scalar
                eng.dma_start_transpose(
                    xqt[:].rearrange("p (k t) -> p k t", k=NK), xq[:]
                )
                xqts[m] = xqt

            # Emission order = Tile priority: first x blocks early (PE ramp),
            # W DMAs interleaved (pure DMA, no engine contention).
            emit_x(0)
            emit_w(0)
            emit_x(1)
            for k in range(1, NK):
                emit_w(k)
                if 1 + k < NM:
                    emit_x(1 + k)

            # ---- Phase 2: bf16 gemm + postprocess per token block ----------
            for m in range(NM):
                xqt = xqts[m]
                gf = gfs[m]
                psums = []
                for n in range(NN):
                    ps = psum_pool.tile([P, NCHUNK], f32, tag="ps", name=f"ps_{m}_{n}")
                    psums.append(ps)
                for k in range(NK):
                    for n in range(NN):
                        nc.tensor.matmul(
                            psums[n][:],
                            xqt[:, k * P : (k + 1) * P],
                            wq8[:, k, n * NCHUNK : (n + 1) * NCHUNK],
                            start=(k == 0),
                            stop=(k == NK - 1),
                        )

                # out = (relu(acc) * ws/s)^2, per 512-chunk, interleaved so
                # psum bank n frees right after its last matmul
                A = a_pool.tile([P, D_OUT], f32, tag="A", name=f"A_{m}")
                B = b_pool.tile([P, D_OUT], f32, tag="B", name=f"B_{m}")
                for n in range(NN):
                    sl = slice(n * NCHUNK, (n + 1) * NCHUNK)
                    nc.scalar.activation(
                        A[:, sl], psums[n][:], AF.Relu, bias=0.0, scale=gf[:, 0:1]
                    )
                    nc.scalar.activation(B[:, sl], A[:, sl], AF.Square)
                    nc.sync.dma_start(y[m * P : (m + 1) * P, sl], B[:, sl])

    _split_excess_waits(nc)
    return nc


def _w_scale_like_reference(weight: np.ndarray) -> float:
    """mean(|W|) computed with jax on CPU so it is bit-identical to the
    reference's jnp.mean(jnp.abs(weight))."""
    try:
        import jax
        import jax.numpy as jnp

        cpu = jax.devices("cpu")[0]
        with jax.default_device(cpu):
            return float(jnp.mean(jnp.abs(jnp.asarray(weight, dtype=jnp.float32))))
    except Exception:
        return float(np.float32(np.abs(weight).astype(np.float64).mean()))


def _ternarize_host(weight: np.ndarray, w_scale: float):
    """Ternary W^T as bf16, matching the reference's f32 compares:
    w_q = 1[w > 0.5*ws] - 1[w < -0.5*ws]."""
    import ml_dtypes

    w = weight.astype(np.float32, copy=False)
    thr = np.float32(0.5) * np.float32(w_scale)
    wqf = (w > thr).astype(np.float32) - (w < -thr).astype(np.float32)
    return np.ascontiguousarray(wqf.T.astype(ml_dtypes.bfloat16))


def make_in_maps(x: np.ndarray, weight: np.ndarray, w_scale: float | None = None):
    if w_scale is None:
        w_scale = _w_scale_like_reference(weight)
    x_flat = np.ascontiguousarray(
        x.reshape(TOK_TOTAL, D_IN).astype(np.float32, copy=False)
    )
    wqt = _ternarize_host(weight, w_scale)
    return [
        {"xs": x_flat[c * TOK : (c + 1) * TOK, :], "wqt": wqt}
        for c in range(N_CORES)
    ]


def run_on_hw(x: np.ndarray, weight: np.ndarray, trace: bool = False):
    """Compile + execute on the 8 NeuronCores.  Returns (y_full, results)."""
    from concourse.bass_utils import run_bass_kernel_spmd

    if trace:
        _install_ntff_hook()
    w_scale = _w_scale_like_reference(weight)
    nc = build_program(w_scale)
    in_maps = make_in_maps(x, weight, w_scale)
    res = run_bass_kernel_spmd(nc, in_maps, list(range(N_CORES)), trace=trace)
    y_full = np.concatenate(
        [np.asarray(res.results[c]["y"]) for c in range(N_CORES)], axis=0
    ).reshape(x.shape[0], x.shape[1], D_OUT)
    return y_full.astype(np.float32, copy=False), res


def _install_ntff_hook():
    """The agent image's antenv package lacks axon_hooks, so NTFF profiling
    silently degrades.  Recreate the hook module (ctypes into
    libaxon_pjrt.so) so run_bass_kernel_spmd(trace=True) works."""
    import types, ctypes, contextlib, os

    if "antenv.axon_hooks" in sys.modules:
        return
    so_path = "/opt/axon/libaxon_pjrt.so"
    if not os.path.exists(so_path):
        return
    lib = ctypes.CDLL(so_path)
    if not hasattr(lib, "axon_start_nrt_profile"):
        return
    lib.axon_start_nrt_profile.argtypes = [
        ctypes.POINTER(ctypes.c_int64),
        ctypes.c_size_t,
    ]
    lib.axon_start_nrt_profile.restype = ctypes.c_int64
    lib.axon_stop_nrt_profile.argtypes = [ctypes.c_char_p]
    lib.axon_stop_nrt_profile.restype = ctypes.c_int64

    @contextlib.contextmanager
    def _hook(output_dir, device_ids):
        import jax

        jax.devices()
        if device_ids:
            ids = (ctypes.c_int64 * len(device_ids))(*device_ids)
            rc = lib.axon_start_nrt_profile(ids, len(device_ids))
        else:
            rc = lib.axon_start_nrt_profile(None, 0)
        if rc != 0:
            raise RuntimeError(f"axon_start_nrt_profile rc={rc}")
        try:
            yield
        finally:
            n = lib.axon_stop_nrt_profile(str(output_dir).encode())
            print(f"profile: {n} file(s) written to {output_dir}", file=sys.stderr)

    mod = types.ModuleType("antenv.axon_hooks")
    mod.get_axon_ntff_profile_hook = lambda: _hook
    mod.set_axon_ntff_profile_hook = lambda h: None
    sys.modules["antenv.axon_hooks"] = mod

    # upload_artifacts needs a coo bucket this container doesn't have;
    # degrade to a no-op so trace processing can proceed locally.
    import concourse.bass_utils as bu

    _orig_upload = bu.upload_artifacts

    def _safe_upload(tmpdir):
        try:
            return _orig_upload(tmpdir)
        except Exception as e:
            print(f"upload_artifacts skipped: {e}", file=sys.stderr)
            return tmpdir

    bu.upload_artifacts = _safe_upload


def kernel(x: np.ndarray, weight: np.ndarray) -> np.ndarray:
    y, _ = run_on_hw(x, weight, trace=False)
    return y
